# revision 14
# baseline (speedup 1.0000x reference)
"""Sliding-window GQA attention (B=2,T=2048,D=2048,N=8,K=4,H=256,W=1024) on 8 trn2 cores.

Sharding: batch over 2 (fsdp) x heads over 4 (tp). Core (b, tp) computes 2 q heads /
1 kv head for batch b; partial [T, D] outputs are summed over tp on the host.

v2 (bf16): all matmul operands bf16 (fp32 PSUM accumulation) — same PE stream
rate as float32r but FWL halves weight-load time (phase B was LDW-bound), DMA
bytes halve, and SBUF pressure drops. Activation-table thrash eliminated:
phase A ACT = Square+Rsqrt only, phase B ACT = Exp only (1/den moved to the
DVE reciprocal_approx_fast custom op, output copies all on DVE). Host packs
weights in SBUF layout so each weight tensor is one batched DMA. Half-masked
edge key-blocks (j=i+1, j=i-8) only compute their valid 128-query half.

Per-core device pipeline:
  A: qT/kT = W^T x^T (head-dim on partitions) and v (natural layout), streaming
     x^T by 512-token quarters; fused RMS-norm + RoPE out of PSUM.
  B: per 256-token query pair: logits^T = kT^T qT per 128-key block (window
     blocks only), exp on ACT (no max-subtraction: |logit| <= ~6), triangular
     masks on DVE, denominator + P^T V via PE accumulation, divide via
     DVE approx-reciprocal.
  C: out = pvT^T o_w accumulated over local heads, emitted lagged one pair
     behind phase B so its matmuls fill exp-wait windows.
"""
import os

import numpy as np
import ml_dtypes

import concourse.bacc as bacc
import concourse.mybir as mybir
from concourse.tile import TileContext
from concourse.bass_utils import run_bass_kernel_spmd

try:  # pragma: no cover - profiling hook is optional
    from antenv.axon_hooks import get_axon_ntff_profile_hook  # noqa: F401
except ImportError:
    os.environ.setdefault("BASS_NEVER_TRACE", "1")


F32 = mybir.dt.float32
BF = mybir.dt.bfloat16
AF = mybir.ActivationFunctionType
OP = mybir.AluOpType

B, T, D = 2, 2048, 2048
N, KV, H = 8, 4, 256
WINDOW = 1024
BASE_FREQ = 10000.0
EPS = 1e-6
NB = T // 128          # 16 token blocks
NQ = 4                 # t quarters for projections (512 each)
NPAIR = 8              # query-block pairs (256 tokens each)


def _jlist(i):
    return list(range(max(0, i - 8), i + 2))


def _build():
    nc = bacc.Bacc(None)

    xT = nc.dram_tensor("xT", [128, 16, T], BF, kind="ExternalInput")
    qw = nc.dram_tensor("qw", [128, 2, 16, 256], BF, kind="ExternalInput")
    kw = nc.dram_tensor("kw", [128, 16, 256], BF, kind="ExternalInput")
    vw = nc.dram_tensor("vw", [128, 16, 256], BF, kind="ExternalInput")
    ow = nc.dram_tensor("ow", [128, 2, 2, D], BF, kind="ExternalInput")
    cosT = nc.dram_tensor("cosT", [128, T], F32, kind="ExternalInput")
    sinT = nc.dram_tensor("sinT", [128, T], F32, kind="ExternalInput")
    masks = nc.dram_tensor("masks", [128, 2, 128], BF, kind="ExternalInput")
    scs = nc.dram_tensor("scs", [128, 2, 2], F32, kind="ExternalInput")  # (1+scale)[q/k][hh]
    out = nc.dram_tensor("out", [T, D], BF, kind="ExternalOutput")

    with TileContext(nc) as tc:
        with tc.tile_pool(name="pers", bufs=1) as pers:
            kT_sb = pers.tile([128, 2, T], BF)
            v_sb = pers.tile([128, NB, H], BF)
            qT_sb = pers.tile([128, 2, 2, T], BF)
            kw_sb = pers.tile([128, 16, 256], BF)
            vw_sb = pers.tile([128, 16, 256], BF)
            qw_sbs = [pers.tile([128, 16, 256], BF, name=f"qw{nl}") for nl in range(2)]
            cos_q = [pers.tile([128, 512], F32, name=f"cos{qt}") for qt in range(NQ)]
            sin_q = [pers.tile([128, 512], F32, name=f"sin{qt}") for qt in range(NQ)]
            scs_sb = pers.tile([128, 2, 2], F32)
            ones32 = pers.tile([128, 128], F32)
            ones = pers.tile([128, 128], BF)
            bias_q = pers.tile([128, 1], F32)
            bias_k = pers.tile([128, 1], F32)
            pvT_sb = pers.tile([128, 2, 2, T], BF)
            ow_sb = pers.tile([128, 2, 2, D], BF)
            masks_sb = pers.tile([128, 2, 128], BF)

            # batched weight DMAs ride the second HWDGE ring (ACT) so they
            # don't queue ahead of the x^T stream on the SP ring. Ordered by
            # first use; quarter 0 runs [k, v, q, q] so the PE has ~14us of
            # qw-independent work while the bigger q weights stream in.
            nc.scalar.dma_start(out=kw_sb, in_=kw[:, :, :])
            nc.scalar.dma_start(out=vw_sb, in_=vw[:, :, :])
            nc.scalar.dma_start(out=cos_q[0], in_=cosT[:, 0:512])
            nc.scalar.dma_start(out=sin_q[0], in_=sinT[:, 0:512])
            nc.scalar.dma_start(out=scs_sb, in_=scs[:, :, :])
            for nl in range(2):
                nc.scalar.dma_start(out=qw_sbs[nl], in_=qw[:, nl, :, :])
            for qt in range(1, NQ):
                tq = slice(512 * qt, 512 * (qt + 1))
                nc.scalar.dma_start(out=cos_q[qt], in_=cosT[:, tq])
                nc.scalar.dma_start(out=sin_q[qt], in_=sinT[:, tq])
            nc.scalar.dma_start(out=masks_sb, in_=masks[:, :, :])
            nc.scalar.dma_start(out=ow_sb, in_=ow[:, :, :, :])
            nc.vector.memset(ones32, 1.0)
            nc.vector.tensor_copy(ones, ones32)
            nc.vector.memset(bias_q, float(H * EPS))
            nc.vector.memset(bias_k, EPS)

            # ---------------- Phase A: all projections + rms + rope ----------------
            with tc.tile_pool(name="xs", bufs=8) as xs, \
                 tc.tile_pool(name="ropep", bufs=1) as ropep, \
                 tc.tile_pool(name="psA", bufs=1, space="PSUM") as psum:

                def rope_emit(p0, p1, dst, kind, cs_t, ss_t):
                    # p0/p1: [128, 512] psum (raw proj h-halves); dst: [128, 2, 512] bf16 view
                    sq0 = ropep.tile([128, 512], BF, tag="sq0", bufs=2)
                    sq1 = ropep.tile([128, 512], BF, tag="sq1", bufs=2)
                    nc.scalar.activation(sq0, p0, AF.Square)
                    nc.scalar.activation(sq1, p1, AF.Square)
                    pss = psum.tile([128, 512], F32, tag="pss", bufs=1)
                    nc.tensor.matmul(pss, ones, sq0, start=True, stop=False)
                    nc.tensor.matmul(pss, ones, sq1, start=False, stop=True)
                    rs = ropep.tile([128, 512], F32, tag="rs", bufs=2)
                    if kind == "q":
                        # 1/16 * rsqrt(ss/256 + eps) == 1/sqrt(ss + 256*eps)
                        nc.scalar.activation(rs, pss, AF.Abs_reciprocal_sqrt,
                                             scale=1.0, bias=bias_q)
                    else:
                        nc.scalar.activation(rs, pss, AF.Abs_reciprocal_sqrt,
                                             scale=1.0 / H, bias=bias_k)
                    cs = ropep.tile([128, 512], F32, tag="cs", bufs=2)
                    ss = ropep.tile([128, 512], F32, tag="ss", bufs=2)
                    nc.vector.tensor_tensor(cs, cs_t, rs, OP.mult)
                    nc.vector.tensor_tensor(ss, ss_t, rs, OP.mult)
                    ki = 0 if kind == "q" else 1
                    s0 = scs_sb[:, ki, 0:1]
                    s1 = scs_sb[:, ki, 1:2]
                    t0 = ropep.tile([128, 512], F32, tag="t0", bufs=2)
                    t1 = ropep.tile([128, 512], F32, tag="t1", bufs=2)
                    nc.vector.scalar_tensor_tensor(t0, p0, s0, cs, OP.mult, OP.mult)
                    nc.vector.scalar_tensor_tensor(t1, p1, s1, ss, OP.mult, OP.mult)
                    nc.vector.tensor_tensor(dst[:, 0, :], t0, t1, OP.subtract)
                    t2 = ropep.tile([128, 512], F32, tag="t0", bufs=2)
                    t3 = ropep.tile([128, 512], F32, tag="t1", bufs=2)
                    nc.vector.scalar_tensor_tensor(t2, p1, s1, cs, OP.mult, OP.mult)
                    nc.vector.scalar_tensor_tensor(t3, p0, s0, ss, OP.mult, OP.mult)
                    nc.vector.tensor_tensor(dst[:, 1, :], t2, t3, OP.add)

                for qt in range(NQ):
                    tq = slice(512 * qt, 512 * (qt + 1))
                    # x^T quarter arrives as 4 batched DMAs of 4 d-chunks each
                    # (fewer DMA_DIRECT2D issues on the sync queue).
                    xqs = []
                    for g in range(4):
                        xq = xs.tile([128, 4, 512], BF, tag="xq")
                        nc.sync.dma_start(out=xq, in_=xT[:, slice(4 * g, 4 * g + 4), tq])
                        xqs.append(xq)
                    xts = [xqs[d // 4][:, d % 4, :] for d in range(16)]
                    cs_t = cos_q[qt]
                    ss_t = sin_q[qt]

                    def emit_k():
                        # k h-halves interleaved per d-chunk (separate PSUM banks)
                        # so PE consumption keeps pace with the x^T DMA stream.
                        # NB: accumulation groups must NOT interleave in one bank.
                        pk = [psum.tile([128, 512], F32, tag="pq", bufs=6, name=f"pk{qt}_{hh}")
                              for hh in range(2)]
                        for d in range(16):
                            nc.tensor.matmul(pk[0], kw_sb[:, d, 0:128], xts[d],
                                             start=(d == 0), stop=(d == 15))
                            nc.tensor.matmul(pk[1], kw_sb[:, d, 128:256], xts[d],
                                             start=(d == 0), stop=(d == 15))
                        rope_emit(pk[0], pk[1], kT_sb[:, :, tq], "k", cs_t, ss_t)

                    def emit_q():
                        for nl in range(2):
                            ps = []
                            for hh in range(2):
                                p = psum.tile([128, 512], F32, tag="pq", bufs=6)
                                hs = slice(128 * hh, 128 * (hh + 1))
                                for d in range(16):
                                    nc.tensor.matmul(p, qw_sbs[nl][:, d, hs], xts[d],
                                                     start=(d == 0), stop=(d == 15))
                                ps.append(p)
                            rope_emit(ps[0], ps[1], qT_sb[:, nl, :, tq], "q", cs_t, ss_t)

                    def emit_v():
                        for half in range(2):
                            p = psum.tile([128, 2, H], F32, tag="pva", bufs=1,
                                          name=f"pv{qt}_{half}")
                            tc0 = 4 * qt + 2 * half
                            for sub in range(2):
                                tl = slice(128 * (2 * half + sub), 128 * (2 * half + sub) + 128)
                                for d in range(16):
                                    nc.tensor.matmul(p[:, sub, :], xts[d][:, tl],
                                                     vw_sb[:, d, :],
                                                     start=(d == 0), stop=(d == 15))
                            nc.vector.tensor_copy(v_sb[:, tc0:tc0 + 2, :], p)

                    if qt == 0:
                        # v before q: fills the PE while qw still streams in.
                        emit_k(); emit_v(); emit_q()
                    else:
                        # v last: its PSUM evacuation is a short DVE copy, so
                        # the psA pool frees quickly at the A->B boundary.
                        emit_k(); emit_q(); emit_v()

                # Preload the Exp activation table while phase A's tail drains
                # (the real first exp would otherwise pay the ~1.3us load right
                # when phase B's denominator is waiting on it).
                dummy = ropep.tile([128, 1], BF, tag="dummy", bufs=1)
                nc.scalar.activation(dummy, bias_q, AF.Exp)

            # ---------------- Phases B + C ----------------
            if True:
                with tc.tile_pool(name="expt", bufs=12) as expt, \
                     tc.tile_pool(name="bw", bufs=4) as bw, \
                     tc.tile_pool(name="oc", bufs=3) as oc, \
                     tc.tile_pool(name="psB", bufs=1, space="PSUM") as psumB:

                    def emit_logits_exp(pi):
                        i = 2 * pi
                        q_lo = slice(256 * pi, 256 * pi + 128)   # first query half
                        q_hi = slice(256 * pi + 128, 256 * (pi + 1))
                        q_all = slice(256 * pi, 256 * (pi + 1))
                        js = _jlist(i)
                        ets = {}
                        up = masks_sb[:, 0, :]
                        lo = masks_sb[:, 1, :]
                        for nl in range(2):
                            for k in range(0, len(js), 2):
                                jp = js[k:k + 2]
                                lp = psumB.tile([128, 2, 256], F32, tag="lp", bufs=4)
                                et = expt.tile([128, 2, 256], BF, tag="et")
                                for x2, j in enumerate(jp):
                                    sj = slice(128 * j, 128 * (j + 1))
                                    if j == i + 1:      # keys above all of q_lo
                                        lps, qsl = lp[:, x2, 128:256], q_hi
                                        ecols = slice(128, 256)
                                    elif j == i - 8:    # keys in-window only for q_lo
                                        lps, qsl = lp[:, x2, 0:128], q_lo
                                        ecols = slice(0, 128)
                                    else:
                                        lps, qsl = lp[:, x2, :], q_all
                                        ecols = slice(0, 256)
                                    nc.tensor.matmul(lps, kT_sb[:, 0, sj],
                                                     qT_sb[:, nl, 0, qsl],
                                                     start=True, stop=False)
                                    nc.tensor.matmul(lps, kT_sb[:, 1, sj],
                                                     qT_sb[:, nl, 1, qsl],
                                                     start=False, stop=True)
                                    ets[(nl, j)] = (et, x2, ecols)
                                # uncomputed edge halves hold stale psum; their
                                # exp lands in et cols no consumer ever reads.
                                nc.scalar.activation(et, lp, AF.Exp)
                                for x2, j in enumerate(jp):
                                    if j == i + 1:
                                        nc.vector.tensor_tensor(
                                            et[:, x2, 128:256], et[:, x2, 128:256], up, OP.mult)
                                    elif j == i:
                                        nc.vector.tensor_tensor(
                                            et[:, x2, 0:128], et[:, x2, 0:128], up, OP.mult)
                                    elif j == i - 7:
                                        nc.vector.tensor_tensor(
                                            et[:, x2, 128:256], et[:, x2, 128:256], lo, OP.mult)
                                    elif j == i - 8:
                                        nc.vector.tensor_tensor(
                                            et[:, x2, 0:128], et[:, x2, 0:128], lo, OP.mult)
                        return ets

                    def emit_tail(pi, ets):
                        i = 2 * pi
                        tqs = slice(256 * pi, 256 * (pi + 1))
                        js = _jlist(i)
                        for nl in range(2):
                            pd = psumB.tile([128, 256], F32, tag="pd", bufs=1)
                            for idx, j in enumerate(js):
                                et, x2, ecols = ets[(nl, j)]
                                nc.tensor.matmul(pd[:, ecols], ones, et[:, x2, ecols],
                                                 start=(idx == 0), stop=(idx == len(js) - 1))
                            rc = bw.tile([128, 256], F32, tag="rc")
                            nc.vector.reciprocal_approx_fast(rc, pd)
                            for hh in range(2):
                                pv = psumB.tile([128, 256], F32, tag="pvb", bufs=3)
                                hs = slice(128 * hh, 128 * (hh + 1))
                                for idx, j in enumerate(js):
                                    et, x2, ecols = ets[(nl, j)]
                                    nc.tensor.matmul(pv[:, ecols], v_sb[:, j, hs],
                                                     et[:, x2, ecols],
                                                     start=(idx == 0), stop=(idx == len(js) - 1))
                                nc.vector.tensor_tensor(pvT_sb[:, nl, hh, tqs], pv, rc, OP.mult)

                    def emit_oproj(pi):
                        # output projection for this pair's two token blocks;
                        # one batched out-DMA per token block (the final block
                        # DMAs per-chunk so the end-of-kernel drain is short).
                        for tb in (2 * pi, 2 * pi + 1):
                            ts_ = slice(128 * tb, 128 * (tb + 1))
                            od = oc.tile([128, 4, 512], BF, tag="od", bufs=3)
                            for dt in range(4):
                                dsl = slice(512 * dt, 512 * (dt + 1))
                                po = psumB.tile([128, 512], F32, tag="pvb", bufs=3)
                                step = 0
                                for nl in range(2):
                                    for hh in range(2):
                                        nc.tensor.matmul(po, pvT_sb[:, nl, hh, ts_],
                                                         ow_sb[:, nl, hh, dsl],
                                                         start=(step == 0), stop=(step == 3))
                                        step += 1
                                nc.vector.tensor_copy(od[:, dt, :], po)
                                if tb == 2 * NPAIR - 1:
                                    nc.sync.dma_start(out=out[ts_, dsl], in_=od[:, dt, :])
                            if tb != 2 * NPAIR - 1:
                                nc.sync.dma_start(out=out[ts_, :], in_=od)

                    # o-proj lags one pair behind: its matmuls fill exp-wait
                    # windows and give the ow DMA time to land after phase A.
                    for pi in range(NPAIR):
                        emit_tail(pi, emit_logits_exp(pi))
                        if pi > 0:
                            emit_oproj(pi - 1)
                    emit_oproj(NPAIR - 1)

    nc.compile()
    return nc


_prog = None
last_results = None


def kernel(x, positions, q_w, k_w, v_w, o_w, q_norm_scale, k_norm_scale):
    global _prog, last_results
    x = np.asarray(x); positions = np.asarray(positions)
    q_w = np.asarray(q_w); k_w = np.asarray(k_w); v_w = np.asarray(v_w); o_w = np.asarray(o_w)
    q_norm_scale = np.asarray(q_norm_scale); k_norm_scale = np.asarray(k_norm_scale)

    if _prog is None:
        _prog = _build()
    nc = _prog

    bf = ml_dtypes.bfloat16

    # host-side constants
    j = np.arange(H // 2, dtype=np.float32)
    timescale = (BASE_FREQ ** (2.0 / H * j)).astype(np.float32)

    c = np.arange(128)[:, None]   # key within block (partition)
    r = np.arange(128)[None, :]   # query within block (column)
    up = (c <= r).astype(np.float32)
    lo = (c > r).astype(np.float32)
    masks_np = np.stack([up, lo], axis=1).astype(bf)  # [128, 2, 128]

    scs_np = np.empty((128, 2, 2), np.float32)
    scs_np[:, 0, 0] = 1.0 + q_norm_scale[:128]
    scs_np[:, 0, 1] = 1.0 + q_norm_scale[128:]
    scs_np[:, 1, 0] = 1.0 + k_norm_scale[:128]
    scs_np[:, 1, 1] = 1.0 + k_norm_scale[128:]

    in_maps = []
    for core in range(8):
        b, tp = core // 4, core % 4
        sinu = positions[b].astype(np.float32)[:, None] / timescale[None, :]  # [T, 128]
        qw_h = np.ascontiguousarray(
            q_w[2 * tp:2 * tp + 2].reshape(2, 16, 128, H).transpose(2, 0, 1, 3)).astype(bf)
        kw_h = np.ascontiguousarray(
            k_w[tp].reshape(16, 128, H).transpose(1, 0, 2)).astype(bf)
        vw_h = np.ascontiguousarray(
            v_w[tp].reshape(16, 128, H).transpose(1, 0, 2)).astype(bf)
        ow_h = np.ascontiguousarray(
            o_w[2 * tp:2 * tp + 2].reshape(2, 2, 128, D).transpose(2, 0, 1, 3)).astype(bf)
        # x^T pre-swizzled to SBUF layout [p, d_chunk, t]
        xT_h = np.ascontiguousarray(
            x[b].T.reshape(16, 128, T).transpose(1, 0, 2)).astype(bf)
        in_maps.append({
            "xT": xT_h,
            "qw": qw_h,
            "kw": kw_h,
            "vw": vw_h,
            "ow": ow_h,
            "cosT": np.ascontiguousarray(np.cos(sinu).T).astype(np.float32),
            "sinT": np.ascontiguousarray(np.sin(sinu).T).astype(np.float32),
            "masks": masks_np,
            "scs": scs_np,
        })

    res = run_bass_kernel_spmd(nc, in_maps, core_ids=list(range(8)))
    last_results = res

    out = np.zeros((B, T, D), np.float32)
    for core in range(8):
        out[core // 4] += res.results[core]["out"].astype(np.float32)
    return out


# revision 18
# speedup vs baseline: 1.1929x; 1.1929x over previous
"""Sliding-window GQA attention (B=2,T=2048,D=2048,N=8,K=4,H=256,W=1024) on 8 trn2 cores.

Sharding: batch over 2 (fsdp) x heads over 4 (tp). Core (b, tp) computes 2 q heads /
1 kv head for batch b; partial [T, D] outputs are summed over tp on the host.

v2 (bf16): all matmul operands bf16 (fp32 PSUM accumulation) — same PE stream
rate as float32r but FWL halves weight-load time (phase B was LDW-bound), DMA
bytes halve, and SBUF pressure drops. Activation-table thrash eliminated:
phase A ACT = Square+Rsqrt only, phase B ACT = Exp only (1/den moved to the
DVE reciprocal_approx_fast custom op, output copies all on DVE). Host packs
weights in SBUF layout so each weight tensor is one batched DMA. Half-masked
edge key-blocks (j=i+1, j=i-8) only compute their valid 128-query half.

Per-core device pipeline:
  A: qT/kT = W^T x^T (head-dim on partitions) and v (natural layout), streaming
     x^T by 512-token quarters; fused RMS-norm + RoPE out of PSUM.
  B: per 256-token query pair: logits^T = kT^T qT per 128-key block (window
     blocks only), exp on ACT (no max-subtraction: |logit| <= ~6), triangular
     masks on DVE, denominator + P^T V via PE accumulation, divide via
     DVE approx-reciprocal.
  C: out = pvT^T o_w accumulated over local heads, emitted lagged one pair
     behind phase B so its matmuls fill exp-wait windows.
"""
import os

import numpy as np
import ml_dtypes

import concourse.bacc as bacc
import concourse.mybir as mybir
from concourse.tile import TileContext
from concourse.bass_utils import run_bass_kernel_spmd

try:  # pragma: no cover - profiling hook is optional
    from antenv.axon_hooks import get_axon_ntff_profile_hook  # noqa: F401
except ImportError:
    os.environ.setdefault("BASS_NEVER_TRACE", "1")


F32 = mybir.dt.float32
BF = mybir.dt.bfloat16
AF = mybir.ActivationFunctionType
OP = mybir.AluOpType

B, T, D = 2, 2048, 2048
N, KV, H = 8, 4, 256
WINDOW = 1024
BASE_FREQ = 10000.0
EPS = 1e-6
NB = T // 128          # 16 token blocks
NQ = 4                 # t quarters for projections (512 each)
NPAIR = 8              # query-block pairs (256 tokens each)


def _jlist(i):
    return list(range(max(0, i - 8), i + 2))


def _build():
    nc = bacc.Bacc(None)

    xT = nc.dram_tensor("xT", [128, 16, T], BF, kind="ExternalInput")
    qw = nc.dram_tensor("qw", [128, 2, 16, 256], BF, kind="ExternalInput")
    kw = nc.dram_tensor("kw", [128, 16, 256], BF, kind="ExternalInput")
    vw = nc.dram_tensor("vw", [128, 16, 256], BF, kind="ExternalInput")
    ow = nc.dram_tensor("ow", [128, 2, 2, D], BF, kind="ExternalInput")
    cosT = nc.dram_tensor("cosT", [128, T], F32, kind="ExternalInput")
    sinT = nc.dram_tensor("sinT", [128, T], F32, kind="ExternalInput")
    masks = nc.dram_tensor("masks", [128, 2, 128], BF, kind="ExternalInput")
    scs = nc.dram_tensor("scs", [128, 2, 2], F32, kind="ExternalInput")  # (1+scale)[q/k][hh]
    out = nc.dram_tensor("out", [T, D], BF, kind="ExternalOutput")

    with TileContext(nc) as tc:
        with tc.tile_pool(name="pers", bufs=1) as pers:
            kT_sb = pers.tile([128, 2, T], BF)
            v_sb = pers.tile([128, NB, H], BF)
            qT_sb = pers.tile([128, 2, 2, T], BF)
            kw_sb = pers.tile([128, 16, 256], BF)
            vw_sb = pers.tile([128, 16, 256], BF)
            qw_sbs = [pers.tile([128, 16, 256], BF, name=f"qw{nl}") for nl in range(2)]
            cos_q = [pers.tile([128, 512], F32, name=f"cos{qt}") for qt in range(NQ)]
            sin_q = [pers.tile([128, 512], F32, name=f"sin{qt}") for qt in range(NQ)]
            scs_sb = pers.tile([128, 2, 2], F32)
            ones32 = pers.tile([128, 128], F32)
            ones = pers.tile([128, 128], BF)
            bias_q = pers.tile([128, 1], F32)
            bias_k = pers.tile([128, 1], F32)
            pvT_sb = pers.tile([128, 2, 2, T], BF)
            ow_sb = pers.tile([128, 2, 2, D], BF)
            masks_sb = pers.tile([128, 2, 128], BF)

            # batched weight DMAs ride the second HWDGE ring (ACT) so they
            # don't queue ahead of the x^T stream on the SP ring. Only what
            # quarter 0's k/v need goes out at t=0 (1.5MB) — the rest is
            # emitted behind phase A ACT ops so its issue is deferred and the
            # x^T stream gets the DMA bandwidth at startup. Quarter 0 runs
            # [k, v, q, q] so the PE has qw-independent work while qw streams.
            nc.scalar.dma_start(out=kw_sb, in_=kw[:, :, :])
            nc.scalar.dma_start(out=vw_sb, in_=vw[:, :, :])
            nc.scalar.dma_start(out=cos_q[0], in_=cosT[:, 0:512])
            nc.scalar.dma_start(out=sin_q[0], in_=sinT[:, 0:512])
            nc.scalar.dma_start(out=scs_sb, in_=scs[:, :, :])
            nc.vector.memset(ones32, 1.0)
            nc.vector.tensor_copy(ones, ones32)
            nc.vector.memset(bias_q, float(H * EPS))
            nc.vector.memset(bias_k, EPS)

            # ---------------- Phase A: all projections + rms + rope ----------------
            with tc.tile_pool(name="xs", bufs=8) as xs, \
                 tc.tile_pool(name="ropep", bufs=1) as ropep, \
                 tc.tile_pool(name="psA", bufs=1, space="PSUM") as psum:

                def rope_emit(p0, p1, dst, kind, cs_t, ss_t):
                    # p0/p1: [128, 512] psum (raw proj h-halves); dst: [128, 2, 512] bf16 view
                    sq0 = ropep.tile([128, 512], BF, tag="sq0", bufs=2)
                    sq1 = ropep.tile([128, 512], BF, tag="sq1", bufs=2)
                    nc.scalar.activation(sq0, p0, AF.Square)
                    nc.scalar.activation(sq1, p1, AF.Square)
                    pss = psum.tile([128, 512], F32, tag="pss", bufs=1)
                    nc.tensor.matmul(pss, ones, sq0, start=True, stop=False)
                    nc.tensor.matmul(pss, ones, sq1, start=False, stop=True)
                    rs = ropep.tile([128, 512], F32, tag="rs", bufs=2)
                    if kind == "q":
                        # 1/16 * rsqrt(ss/256 + eps) == 1/sqrt(ss + 256*eps)
                        nc.scalar.activation(rs, pss, AF.Abs_reciprocal_sqrt,
                                             scale=1.0, bias=bias_q)
                    else:
                        nc.scalar.activation(rs, pss, AF.Abs_reciprocal_sqrt,
                                             scale=1.0 / H, bias=bias_k)
                    cs = ropep.tile([128, 512], F32, tag="cs", bufs=2)
                    ss = ropep.tile([128, 512], F32, tag="ss", bufs=2)
                    nc.vector.tensor_tensor(cs, cs_t, rs, OP.mult)
                    nc.vector.tensor_tensor(ss, ss_t, rs, OP.mult)
                    ki = 0 if kind == "q" else 1
                    s0 = scs_sb[:, ki, 0:1]
                    s1 = scs_sb[:, ki, 1:2]
                    t0 = ropep.tile([128, 512], F32, tag="t0", bufs=2)
                    t1 = ropep.tile([128, 512], F32, tag="t1", bufs=2)
                    nc.vector.scalar_tensor_tensor(t0, p0, s0, cs, OP.mult, OP.mult)
                    nc.vector.scalar_tensor_tensor(t1, p1, s1, ss, OP.mult, OP.mult)
                    nc.vector.tensor_tensor(dst[:, 0, :], t0, t1, OP.subtract)
                    t2 = ropep.tile([128, 512], F32, tag="t0", bufs=2)
                    t3 = ropep.tile([128, 512], F32, tag="t1", bufs=2)
                    nc.vector.scalar_tensor_tensor(t2, p1, s1, cs, OP.mult, OP.mult)
                    nc.vector.scalar_tensor_tensor(t3, p0, s0, ss, OP.mult, OP.mult)
                    nc.vector.tensor_tensor(dst[:, 1, :], t2, t3, OP.add)
                    return rs

                for qt in range(NQ):
                    tq = slice(512 * qt, 512 * (qt + 1))
                    # x^T quarter arrives as 4 batched DMAs of 4 d-chunks each
                    # (fewer DMA_DIRECT2D issues on the sync queue).
                    xqs = []
                    for g in range(4):
                        xq = xs.tile([128, 4, 512], BF, tag="xq")
                        nc.sync.dma_start(out=xq, in_=xT[:, slice(4 * g, 4 * g + 4), tq])
                        xqs.append(xq)
                    xts = [xqs[d // 4][:, d % 4, :] for d in range(16)]
                    cs_t = cos_q[qt]
                    ss_t = sin_q[qt]

                    def emit_k():
                        # k h-halves interleaved per d-chunk (separate PSUM banks)
                        # so PE consumption keeps pace with the x^T DMA stream.
                        # NB: accumulation groups must NOT interleave in one bank.
                        pk = [psum.tile([128, 512], F32, tag="pq", bufs=6, name=f"pk{qt}_{hh}")
                              for hh in range(2)]
                        for d in range(16):
                            nc.tensor.matmul(pk[0], kw_sb[:, d, 0:128], xts[d],
                                             start=(d == 0), stop=(d == 15))
                            nc.tensor.matmul(pk[1], kw_sb[:, d, 128:256], xts[d],
                                             start=(d == 0), stop=(d == 15))
                        return rope_emit(pk[0], pk[1], kT_sb[:, :, tq], "k", cs_t, ss_t)

                    def emit_q():
                        for nl in range(2):
                            ps = []
                            for hh in range(2):
                                p = psum.tile([128, 512], F32, tag="pq", bufs=6)
                                hs = slice(128 * hh, 128 * (hh + 1))
                                for d in range(16):
                                    nc.tensor.matmul(p, qw_sbs[nl][:, d, hs], xts[d],
                                                     start=(d == 0), stop=(d == 15))
                                ps.append(p)
                            rs = rope_emit(ps[0], ps[1], qT_sb[:, nl, :, tq], "q", cs_t, ss_t)
                        return rs

                    def emit_v():
                        for half in range(2):
                            p = psum.tile([128, 2, H], F32, tag="pva", bufs=1,
                                          name=f"pv{qt}_{half}")
                            tc0 = 4 * qt + 2 * half
                            for sub in range(2):
                                tl = slice(128 * (2 * half + sub), 128 * (2 * half + sub) + 128)
                                for d in range(16):
                                    nc.tensor.matmul(p[:, sub, :], xts[d][:, tl],
                                                     vw_sb[:, d, :],
                                                     start=(d == 0), stop=(d == 15))
                            nc.vector.tensor_copy(v_sb[:, tc0:tc0 + 2, :], p)

                    if qt == 0:
                        # v before q: fills the PE while qw still streams in.
                        # The deferred DMA issues sit behind k's rope ACTs on
                        # the scalar queue, so they don't steal bandwidth from
                        # the quarter-0 x^T stream.
                        emit_k()
                        for nl in range(2):
                            nc.scalar.dma_start(out=qw_sbs[nl], in_=qw[:, nl, :, :])
                        emit_v()
                        nc.scalar.dma_start(out=cos_q[1], in_=cosT[:, 512:1024])
                        nc.scalar.dma_start(out=sin_q[1], in_=sinT[:, 512:1024])
                        nc.scalar.dma_start(out=masks_sb, in_=masks[:, :, :])
                        last_rs = emit_q()
                    elif qt == 1:
                        emit_k()
                        nc.scalar.dma_start(out=cos_q[2], in_=cosT[:, 1024:1536])
                        nc.scalar.dma_start(out=sin_q[2], in_=sinT[:, 1024:1536])
                        nc.scalar.dma_start(out=ow_sb, in_=ow[:, :, :, :])
                        last_rs = emit_q()
                        emit_v()
                    elif qt == 2:
                        emit_k()
                        nc.scalar.dma_start(out=cos_q[3], in_=cosT[:, 1536:2048])
                        nc.scalar.dma_start(out=sin_q[3], in_=sinT[:, 1536:2048])
                        last_rs = emit_q()
                        emit_v()
                    else:
                        # v last: its PSUM evacuation is a short DVE copy, so
                        # the psA pool frees quickly at the A->B boundary.
                        emit_k()
                        last_rs = emit_q()
                        emit_v()

                # Preload the Exp activation table while phase A's tail drains
                # (the real first exp would otherwise pay the ~1.3us load right
                # when phase B's denominator is waiting on it). Reading the last
                # rope rsqrt output pins this after all phase A ACT work.
                dummy = ropep.tile([128, 1], BF, tag="dummy", bufs=1)
                nc.scalar.activation(dummy, last_rs[:, 0:1], AF.Exp)

            # ---------------- Phases B + C ----------------
            if True:
                with tc.tile_pool(name="expt", bufs=12) as expt, \
                     tc.tile_pool(name="bw", bufs=4) as bw, \
                     tc.tile_pool(name="oc", bufs=3) as oc, \
                     tc.tile_pool(name="psB", bufs=1, space="PSUM") as psumB:

                    def emit_logits_exp(pi):
                        i = 2 * pi
                        q_lo = slice(256 * pi, 256 * pi + 128)   # first query half
                        q_hi = slice(256 * pi + 128, 256 * (pi + 1))
                        q_all = slice(256 * pi, 256 * (pi + 1))
                        js = _jlist(i)
                        ets = {}
                        up = masks_sb[:, 0, :]
                        lo = masks_sb[:, 1, :]
                        for nl in range(2):
                            for k in range(0, len(js), 2):
                                jp = js[k:k + 2]
                                lp = psumB.tile([128, 2, 256], F32, tag="lp", bufs=4)
                                et = expt.tile([128, 2, 256], BF, tag="et")
                                for x2, j in enumerate(jp):
                                    sj = slice(128 * j, 128 * (j + 1))
                                    if j == i + 1:      # keys above all of q_lo
                                        lps, qsl = lp[:, x2, 128:256], q_hi
                                        ecols = slice(128, 256)
                                    elif j == i - 8:    # keys in-window only for q_lo
                                        lps, qsl = lp[:, x2, 0:128], q_lo
                                        ecols = slice(0, 128)
                                    else:
                                        lps, qsl = lp[:, x2, :], q_all
                                        ecols = slice(0, 256)
                                    nc.tensor.matmul(lps, kT_sb[:, 0, sj],
                                                     qT_sb[:, nl, 0, qsl],
                                                     start=True, stop=False)
                                    nc.tensor.matmul(lps, kT_sb[:, 1, sj],
                                                     qT_sb[:, nl, 1, qsl],
                                                     start=False, stop=True)
                                    ets[(nl, j)] = (et, x2, ecols)
                                # uncomputed edge halves hold stale psum; their
                                # exp lands in et cols no consumer ever reads.
                                nc.scalar.activation(et, lp, AF.Exp)
                                for x2, j in enumerate(jp):
                                    if j == i + 1:
                                        nc.vector.tensor_tensor(
                                            et[:, x2, 128:256], et[:, x2, 128:256], up, OP.mult)
                                    elif j == i:
                                        nc.vector.tensor_tensor(
                                            et[:, x2, 0:128], et[:, x2, 0:128], up, OP.mult)
                                    elif j == i - 7:
                                        nc.vector.tensor_tensor(
                                            et[:, x2, 128:256], et[:, x2, 128:256], lo, OP.mult)
                                    elif j == i - 8:
                                        nc.vector.tensor_tensor(
                                            et[:, x2, 0:128], et[:, x2, 0:128], lo, OP.mult)
                        return ets

                    def emit_tail(pi, ets):
                        i = 2 * pi
                        tqs = slice(256 * pi, 256 * (pi + 1))
                        js = _jlist(i)
                        for nl in range(2):
                            pd = psumB.tile([128, 256], F32, tag="pd", bufs=1)
                            for idx, j in enumerate(js):
                                et, x2, ecols = ets[(nl, j)]
                                nc.tensor.matmul(pd[:, ecols], ones, et[:, x2, ecols],
                                                 start=(idx == 0), stop=(idx == len(js) - 1))
                            rc = bw.tile([128, 256], F32, tag="rc")
                            nc.vector.reciprocal_approx_fast(rc, pd)
                            for hh in range(2):
                                pv = psumB.tile([128, 256], F32, tag="pvb", bufs=3)
                                hs = slice(128 * hh, 128 * (hh + 1))
                                for idx, j in enumerate(js):
                                    et, x2, ecols = ets[(nl, j)]
                                    nc.tensor.matmul(pv[:, ecols], v_sb[:, j, hs],
                                                     et[:, x2, ecols],
                                                     start=(idx == 0), stop=(idx == len(js) - 1))
                                nc.vector.tensor_tensor(pvT_sb[:, nl, hh, tqs], pv, rc, OP.mult)

                    def emit_oproj(pi):
                        # output projection for this pair's two token blocks;
                        # one batched out-DMA per token block (the final block
                        # DMAs per-chunk so the end-of-kernel drain is short).
                        for tb in (2 * pi, 2 * pi + 1):
                            ts_ = slice(128 * tb, 128 * (tb + 1))
                            od = oc.tile([128, 4, 512], BF, tag="od", bufs=3)
                            for dt in range(4):
                                dsl = slice(512 * dt, 512 * (dt + 1))
                                po = psumB.tile([128, 512], F32, tag="pvb", bufs=3)
                                step = 0
                                for nl in range(2):
                                    for hh in range(2):
                                        nc.tensor.matmul(po, pvT_sb[:, nl, hh, ts_],
                                                         ow_sb[:, nl, hh, dsl],
                                                         start=(step == 0), stop=(step == 3))
                                        step += 1
                                nc.vector.tensor_copy(od[:, dt, :], po)
                                if tb == 2 * NPAIR - 1:
                                    nc.sync.dma_start(out=out[ts_, dsl], in_=od[:, dt, :])
                            if tb != 2 * NPAIR - 1:
                                nc.sync.dma_start(out=out[ts_, :], in_=od)

                    # o-proj lags one pair behind: its matmuls fill exp-wait
                    # windows and give the ow DMA time to land after phase A.
                    for pi in range(NPAIR):
                        emit_tail(pi, emit_logits_exp(pi))
                        if pi > 0:
                            emit_oproj(pi - 1)
                    emit_oproj(NPAIR - 1)

    nc.compile()
    return nc


_prog = None
last_results = None


def kernel(x, positions, q_w, k_w, v_w, o_w, q_norm_scale, k_norm_scale):
    global _prog, last_results
    x = np.asarray(x); positions = np.asarray(positions)
    q_w = np.asarray(q_w); k_w = np.asarray(k_w); v_w = np.asarray(v_w); o_w = np.asarray(o_w)
    q_norm_scale = np.asarray(q_norm_scale); k_norm_scale = np.asarray(k_norm_scale)

    if _prog is None:
        _prog = _build()
    nc = _prog

    bf = ml_dtypes.bfloat16

    # host-side constants
    j = np.arange(H // 2, dtype=np.float32)
    timescale = (BASE_FREQ ** (2.0 / H * j)).astype(np.float32)

    c = np.arange(128)[:, None]   # key within block (partition)
    r = np.arange(128)[None, :]   # query within block (column)
    up = (c <= r).astype(np.float32)
    lo = (c > r).astype(np.float32)
    masks_np = np.stack([up, lo], axis=1).astype(bf)  # [128, 2, 128]

    scs_np = np.empty((128, 2, 2), np.float32)
    scs_np[:, 0, 0] = 1.0 + q_norm_scale[:128]
    scs_np[:, 0, 1] = 1.0 + q_norm_scale[128:]
    scs_np[:, 1, 0] = 1.0 + k_norm_scale[:128]
    scs_np[:, 1, 1] = 1.0 + k_norm_scale[128:]

    in_maps = []
    for core in range(8):
        b, tp = core // 4, core % 4
        sinu = positions[b].astype(np.float32)[:, None] / timescale[None, :]  # [T, 128]
        qw_h = np.ascontiguousarray(
            q_w[2 * tp:2 * tp + 2].reshape(2, 16, 128, H).transpose(2, 0, 1, 3)).astype(bf)
        kw_h = np.ascontiguousarray(
            k_w[tp].reshape(16, 128, H).transpose(1, 0, 2)).astype(bf)
        vw_h = np.ascontiguousarray(
            v_w[tp].reshape(16, 128, H).transpose(1, 0, 2)).astype(bf)
        ow_h = np.ascontiguousarray(
            o_w[2 * tp:2 * tp + 2].reshape(2, 2, 128, D).transpose(2, 0, 1, 3)).astype(bf)
        # x^T pre-swizzled to SBUF layout [p, d_chunk, t]
        xT_h = np.ascontiguousarray(
            x[b].T.reshape(16, 128, T).transpose(1, 0, 2)).astype(bf)
        in_maps.append({
            "xT": xT_h,
            "qw": qw_h,
            "kw": kw_h,
            "vw": vw_h,
            "ow": ow_h,
            "cosT": np.ascontiguousarray(np.cos(sinu).T).astype(np.float32),
            "sinT": np.ascontiguousarray(np.sin(sinu).T).astype(np.float32),
            "masks": masks_np,
            "scs": scs_np,
        })

    res = run_bass_kernel_spmd(nc, in_maps, core_ids=list(range(8)))
    last_results = res

    out = np.zeros((B, T, D), np.float32)
    for core in range(8):
        out[core // 4] += res.results[core]["out"].astype(np.float32)
    return out


# revision 22
# speedup vs baseline: 1.2207x; 1.0233x over previous
"""Sliding-window GQA attention (B=2,T=2048,D=2048,N=8,K=4,H=256,W=1024) on 8 trn2 cores.

Sharding: batch over 2 (fsdp) x heads over 4 (tp). Core (b, tp) computes 2 q heads /
1 kv head for batch b; partial [T, D] outputs are summed over tp on the host.

v2 (bf16): all matmul operands bf16 (fp32 PSUM accumulation) — same PE stream
rate as float32r but FWL halves weight-load time (phase B was LDW-bound), DMA
bytes halve, and SBUF pressure drops. Activation-table thrash eliminated:
phase A ACT = Square+Rsqrt only, phase B ACT = Exp only (1/den moved to the
DVE reciprocal_approx_fast custom op, output copies all on DVE). Host packs
weights in SBUF layout so each weight tensor is one batched DMA. Half-masked
edge key-blocks (j=i+1, j=i-8) only compute their valid 128-query half.

Per-core device pipeline:
  A: qT/kT = W^T x^T (head-dim on partitions) and v (natural layout), streaming
     x^T by 512-token quarters; fused RMS-norm + RoPE out of PSUM.
  B: per 256-token query pair: logits^T = kT^T qT per 128-key block (window
     blocks only), exp on ACT (no max-subtraction: |logit| <= ~6), triangular
     masks on DVE, denominator + P^T V via PE accumulation, divide via
     DVE approx-reciprocal.
  C: out = pvT^T o_w accumulated over local heads, emitted lagged one pair
     behind phase B so its matmuls fill exp-wait windows.
"""
import os

import numpy as np
import ml_dtypes

import concourse.bacc as bacc
import concourse.mybir as mybir
from concourse.tile import TileContext
from concourse.bass_utils import run_bass_kernel_spmd

try:  # pragma: no cover - profiling hook is optional
    from antenv.axon_hooks import get_axon_ntff_profile_hook  # noqa: F401
except ImportError:
    os.environ.setdefault("BASS_NEVER_TRACE", "1")


F32 = mybir.dt.float32
BF = mybir.dt.bfloat16
AF = mybir.ActivationFunctionType
OP = mybir.AluOpType

B, T, D = 2, 2048, 2048
N, KV, H = 8, 4, 256
WINDOW = 1024
BASE_FREQ = 10000.0
EPS = 1e-6
NB = T // 128          # 16 token blocks
NQ = 4                 # t quarters for projections (512 each)
NPAIR = 8              # query-block pairs (256 tokens each)


def _jlist(i):
    return list(range(max(0, i - 8), i + 2))


def _build():
    nc = bacc.Bacc(None)

    xT = nc.dram_tensor("xT", [128, NQ, 16, 512], BF, kind="ExternalInput")
    qw = nc.dram_tensor("qw", [128, 2, 16, 256], BF, kind="ExternalInput")
    kw = nc.dram_tensor("kw", [128, 16, 256], BF, kind="ExternalInput")
    vw = nc.dram_tensor("vw", [128, 16, 256], BF, kind="ExternalInput")
    ow = nc.dram_tensor("ow", [128, 2, 2, D], BF, kind="ExternalInput")
    cosT = nc.dram_tensor("cosT", [128, T], F32, kind="ExternalInput")
    sinT = nc.dram_tensor("sinT", [128, T], F32, kind="ExternalInput")
    masks = nc.dram_tensor("masks", [128, 2, 128], BF, kind="ExternalInput")
    scs = nc.dram_tensor("scs", [128, 2, 2], F32, kind="ExternalInput")  # (1+scale)[q/k][hh]
    out = nc.dram_tensor("out", [T, D], BF, kind="ExternalOutput")

    with TileContext(nc) as tc:
        with tc.tile_pool(name="pers", bufs=1) as pers:
            kT_sb = pers.tile([128, 2, T], BF)
            v_sb = pers.tile([128, NB, H], BF)
            qT_sb = pers.tile([128, 2, 2, T], BF)
            kw_sb = pers.tile([128, 16, 256], BF)
            vw_sb = pers.tile([128, 16, 256], BF)
            qw_sbs = [pers.tile([128, 16, 256], BF, name=f"qw{nl}") for nl in range(2)]
            cos_q = [pers.tile([128, 512], F32, name=f"cos{qt}") for qt in range(NQ)]
            sin_q = [pers.tile([128, 512], F32, name=f"sin{qt}") for qt in range(NQ)]
            scs_sb = pers.tile([128, 2, 2], F32)
            ones32 = pers.tile([128, 128], F32)
            ones = pers.tile([128, 128], BF)
            bias_q = pers.tile([128, 1], F32)
            bias_k = pers.tile([128, 1], F32)
            pvT_sb = pers.tile([128, 2, 2, T], BF)
            ow_sb = pers.tile([128, 2, 2, D], BF)
            masks_sb = pers.tile([128, 2, 128], BF)

            # batched weight DMAs ride the second HWDGE ring (ACT) so they
            # don't queue ahead of the x^T stream on the SP ring. Only what
            # quarter 0's k/v need goes out at t=0 (1.5MB) — the rest is
            # emitted behind phase A ACT ops so its issue is deferred and the
            # x^T stream gets the DMA bandwidth at startup. Quarter 0 runs
            # [k, v, q, q] so the PE has qw-independent work while qw streams.
            nc.scalar.dma_start(out=kw_sb, in_=kw[:, :, :])
            nc.scalar.dma_start(out=vw_sb, in_=vw[:, :, :])
            nc.scalar.dma_start(out=cos_q[0], in_=cosT[:, 0:512])
            nc.scalar.dma_start(out=sin_q[0], in_=sinT[:, 0:512])
            nc.scalar.dma_start(out=scs_sb, in_=scs[:, :, :])
            nc.vector.memset(ones32, 1.0)
            nc.vector.tensor_copy(ones, ones32)
            nc.vector.memset(bias_q, float(H * EPS))
            nc.vector.memset(bias_k, EPS)

            # ---------------- Phase A: all projections + rms + rope ----------------
            with tc.tile_pool(name="xs", bufs=8) as xs, \
                 tc.tile_pool(name="ropep", bufs=1) as ropep, \
                 tc.tile_pool(name="psA", bufs=1, space="PSUM") as psum:

                def rope_emit(p0, p1, dst, kind, cs_t, ss_t):
                    # p0/p1: [128, 512] psum (raw proj h-halves); dst: [128, 2, 512] bf16 view
                    sq0 = ropep.tile([128, 512], BF, tag="sq0", bufs=2)
                    sq1 = ropep.tile([128, 512], BF, tag="sq1", bufs=2)
                    nc.scalar.activation(sq0, p0, AF.Square)
                    nc.scalar.activation(sq1, p1, AF.Square)
                    pss = psum.tile([128, 512], F32, tag="pss", bufs=1)
                    nc.tensor.matmul(pss, ones, sq0, start=True, stop=False)
                    nc.tensor.matmul(pss, ones, sq1, start=False, stop=True)
                    rs = ropep.tile([128, 512], F32, tag="rs", bufs=2)
                    if kind == "q":
                        # 1/16 * rsqrt(ss/256 + eps) == 1/sqrt(ss + 256*eps)
                        nc.scalar.activation(rs, pss, AF.Abs_reciprocal_sqrt,
                                             scale=1.0, bias=bias_q)
                    else:
                        nc.scalar.activation(rs, pss, AF.Abs_reciprocal_sqrt,
                                             scale=1.0 / H, bias=bias_k)
                    cs = ropep.tile([128, 512], F32, tag="cs", bufs=2)
                    ss = ropep.tile([128, 512], F32, tag="ss", bufs=2)
                    nc.vector.tensor_tensor(cs, cs_t, rs, OP.mult)
                    nc.vector.tensor_tensor(ss, ss_t, rs, OP.mult)
                    ki = 0 if kind == "q" else 1
                    s0 = scs_sb[:, ki, 0:1]
                    s1 = scs_sb[:, ki, 1:2]
                    t0 = ropep.tile([128, 512], F32, tag="t0", bufs=2)
                    t1 = ropep.tile([128, 512], F32, tag="t1", bufs=2)
                    nc.vector.scalar_tensor_tensor(t0, p0, s0, cs, OP.mult, OP.mult)
                    nc.vector.scalar_tensor_tensor(t1, p1, s1, ss, OP.mult, OP.mult)
                    nc.vector.tensor_tensor(dst[:, 0, :], t0, t1, OP.subtract)
                    t2 = ropep.tile([128, 512], F32, tag="t0", bufs=2)
                    t3 = ropep.tile([128, 512], F32, tag="t1", bufs=2)
                    nc.vector.scalar_tensor_tensor(t2, p1, s1, cs, OP.mult, OP.mult)
                    nc.vector.scalar_tensor_tensor(t3, p0, s0, ss, OP.mult, OP.mult)
                    nc.vector.tensor_tensor(dst[:, 1, :], t2, t3, OP.add)
                    return rs

                for qt in range(NQ):
                    tq = slice(512 * qt, 512 * (qt + 1))
                    # x^T quarter arrives as 4 batched DMAs of 4 d-chunks each
                    # (fewer DMA_DIRECT2D issues on the sync queue).
                    xqs = []
                    for g in range(4):
                        xq = xs.tile([128, 4, 512], BF, tag="xq")
                        # quarter-major host layout: each group is a contiguous
                        # 4KB run per partition (big DMA descriptors win a fair
                        # round-robin share against the weight ring's packets)
                        nc.sync.dma_start(out=xq, in_=xT[:, qt, slice(4 * g, 4 * g + 4), :])
                        xqs.append(xq)
                    xts = [xqs[d // 4][:, d % 4, :] for d in range(16)]
                    cs_t = cos_q[qt]
                    ss_t = sin_q[qt]

                    def emit_k():
                        # k h-halves interleaved per d-chunk (separate PSUM banks)
                        # so PE consumption keeps pace with the x^T DMA stream.
                        # NB: accumulation groups must NOT interleave in one bank.
                        pk = [psum.tile([128, 512], F32, tag="pq", bufs=6, name=f"pk{qt}_{hh}")
                              for hh in range(2)]
                        for d in range(16):
                            nc.tensor.matmul(pk[0], kw_sb[:, d, 0:128], xts[d],
                                             start=(d == 0), stop=(d == 15))
                            nc.tensor.matmul(pk[1], kw_sb[:, d, 128:256], xts[d],
                                             start=(d == 0), stop=(d == 15))
                        return rope_emit(pk[0], pk[1], kT_sb[:, :, tq], "k", cs_t, ss_t)

                    def emit_q():
                        for nl in range(2):
                            ps = []
                            for hh in range(2):
                                p = psum.tile([128, 512], F32, tag="pq", bufs=6)
                                hs = slice(128 * hh, 128 * (hh + 1))
                                for d in range(16):
                                    nc.tensor.matmul(p, qw_sbs[nl][:, d, hs], xts[d],
                                                     start=(d == 0), stop=(d == 15))
                                ps.append(p)
                            rs = rope_emit(ps[0], ps[1], qT_sb[:, nl, :, tq], "q", cs_t, ss_t)
                        return rs

                    def emit_v():
                        for half in range(2):
                            p = psum.tile([128, 2, H], F32, tag="pva", bufs=1,
                                          name=f"pv{qt}_{half}")
                            tc0 = 4 * qt + 2 * half
                            for sub in range(2):
                                tl = slice(128 * (2 * half + sub), 128 * (2 * half + sub) + 128)
                                for d in range(16):
                                    nc.tensor.matmul(p[:, sub, :], xts[d][:, tl],
                                                     vw_sb[:, d, :],
                                                     start=(d == 0), stop=(d == 15))
                            if qt == NQ - 1:
                                # ACT copy: the DVE queue still holds the last
                                # rope chain; this frees psA sooner at A->B.
                                nc.scalar.copy(v_sb[:, tc0:tc0 + 2, :], p)
                            else:
                                nc.vector.tensor_copy(v_sb[:, tc0:tc0 + 2, :], p)

                    if qt == 0:
                        # v before q: fills the PE while qw still streams in.
                        # The deferred DMA issues sit behind k's rope ACTs on
                        # the scalar queue, so they don't steal bandwidth from
                        # the quarter-0 x^T stream.
                        emit_k()
                        for nl in range(2):
                            nc.scalar.dma_start(out=qw_sbs[nl], in_=qw[:, nl, :, :])
                        emit_v()
                        nc.scalar.dma_start(out=cos_q[1], in_=cosT[:, 512:1024])
                        nc.scalar.dma_start(out=sin_q[1], in_=sinT[:, 512:1024])
                        nc.scalar.dma_start(out=masks_sb, in_=masks[:, :, :])
                        last_rs = emit_q()
                    elif qt == 1:
                        emit_k()
                        nc.scalar.dma_start(out=cos_q[2], in_=cosT[:, 1024:1536])
                        nc.scalar.dma_start(out=sin_q[2], in_=sinT[:, 1024:1536])
                        nc.scalar.dma_start(out=ow_sb, in_=ow[:, :, :, :])
                        last_rs = emit_q()
                        emit_v()
                    elif qt == 2:
                        emit_k()
                        nc.scalar.dma_start(out=cos_q[3], in_=cosT[:, 1536:2048])
                        nc.scalar.dma_start(out=sin_q[3], in_=sinT[:, 1536:2048])
                        last_rs = emit_q()
                        emit_v()
                    else:
                        # v last: its PSUM evacuation is a short DVE copy, so
                        # the psA pool frees quickly at the A->B boundary.
                        emit_k()
                        last_rs = emit_q()
                        emit_v()

                # Preload the Exp activation table while phase A's tail drains
                # (the real first exp would otherwise pay the ~1.3us load right
                # when phase B's denominator is waiting on it). Reading the last
                # rope rsqrt output pins this after all phase A ACT work.
                dummy = ropep.tile([128, 1], BF, tag="dummy", bufs=1)
                nc.scalar.activation(dummy, last_rs[:, 0:1], AF.Exp)

            # ---------------- Phases B + C ----------------
            if True:
                with tc.tile_pool(name="expt", bufs=12) as expt, \
                     tc.tile_pool(name="bw", bufs=4) as bw, \
                     tc.tile_pool(name="oc", bufs=3) as oc, \
                     tc.tile_pool(name="psB", bufs=1, space="PSUM") as psumB:

                    def emit_logits_exp(pi):
                        i = 2 * pi
                        q_lo = slice(256 * pi, 256 * pi + 128)   # first query half
                        q_hi = slice(256 * pi + 128, 256 * (pi + 1))
                        q_all = slice(256 * pi, 256 * (pi + 1))
                        js = _jlist(i)
                        ets = {}
                        up = masks_sb[:, 0, :]
                        lo = masks_sb[:, 1, :]
                        for nl in range(2):
                            for k in range(0, len(js), 2):
                                jp = js[k:k + 2]
                                lp = psumB.tile([128, 2, 256], F32, tag="lp", bufs=4)
                                et = expt.tile([128, 2, 256], BF, tag="et")
                                for x2, j in enumerate(jp):
                                    sj = slice(128 * j, 128 * (j + 1))
                                    if j == i + 1:      # keys above all of q_lo
                                        lps, qsl = lp[:, x2, 128:256], q_hi
                                        ecols = slice(128, 256)
                                    elif j == i - 8:    # keys in-window only for q_lo
                                        lps, qsl = lp[:, x2, 0:128], q_lo
                                        ecols = slice(0, 128)
                                    else:
                                        lps, qsl = lp[:, x2, :], q_all
                                        ecols = slice(0, 256)
                                    nc.tensor.matmul(lps, kT_sb[:, 0, sj],
                                                     qT_sb[:, nl, 0, qsl],
                                                     start=True, stop=False)
                                    nc.tensor.matmul(lps, kT_sb[:, 1, sj],
                                                     qT_sb[:, nl, 1, qsl],
                                                     start=False, stop=True)
                                    ets[(nl, j)] = (et, x2, ecols)
                                # uncomputed edge halves hold stale psum; their
                                # exp lands in et cols no consumer ever reads.
                                nc.scalar.activation(et, lp, AF.Exp)
                                for x2, j in enumerate(jp):
                                    if j == i + 1:
                                        nc.vector.tensor_tensor(
                                            et[:, x2, 128:256], et[:, x2, 128:256], up, OP.mult)
                                    elif j == i:
                                        nc.vector.tensor_tensor(
                                            et[:, x2, 0:128], et[:, x2, 0:128], up, OP.mult)
                                    elif j == i - 7:
                                        nc.vector.tensor_tensor(
                                            et[:, x2, 128:256], et[:, x2, 128:256], lo, OP.mult)
                                    elif j == i - 8:
                                        nc.vector.tensor_tensor(
                                            et[:, x2, 0:128], et[:, x2, 0:128], lo, OP.mult)
                        return ets

                    def emit_tail(pi, ets):
                        i = 2 * pi
                        tqs = slice(256 * pi, 256 * (pi + 1))
                        js = _jlist(i)
                        for nl in range(2):
                            pd = psumB.tile([128, 256], F32, tag="pd", bufs=1)
                            for idx, j in enumerate(js):
                                et, x2, ecols = ets[(nl, j)]
                                nc.tensor.matmul(pd[:, ecols], ones, et[:, x2, ecols],
                                                 start=(idx == 0), stop=(idx == len(js) - 1))
                            rc = bw.tile([128, 256], F32, tag="rc")
                            nc.vector.reciprocal_approx_fast(rc, pd)
                            for hh in range(2):
                                pv = psumB.tile([128, 256], F32, tag="pvb", bufs=3)
                                hs = slice(128 * hh, 128 * (hh + 1))
                                for idx, j in enumerate(js):
                                    et, x2, ecols = ets[(nl, j)]
                                    nc.tensor.matmul(pv[:, ecols], v_sb[:, j, hs],
                                                     et[:, x2, ecols],
                                                     start=(idx == 0), stop=(idx == len(js) - 1))
                                nc.vector.tensor_tensor(pvT_sb[:, nl, hh, tqs], pv, rc, OP.mult)

                    def emit_oproj(pi):
                        # output projection for this pair's two token blocks;
                        # one batched out-DMA per token block (the final block
                        # DMAs per-chunk so the end-of-kernel drain is short).
                        for tb in (2 * pi, 2 * pi + 1):
                            ts_ = slice(128 * tb, 128 * (tb + 1))
                            od = oc.tile([128, 4, 512], BF, tag="od", bufs=3)
                            for dt in range(4):
                                dsl = slice(512 * dt, 512 * (dt + 1))
                                po = psumB.tile([128, 512], F32, tag="pvb", bufs=3)
                                step = 0
                                for nl in range(2):
                                    for hh in range(2):
                                        nc.tensor.matmul(po, pvT_sb[:, nl, hh, ts_],
                                                         ow_sb[:, nl, hh, dsl],
                                                         start=(step == 0), stop=(step == 3))
                                        step += 1
                                nc.vector.tensor_copy(od[:, dt, :], po)
                                if tb == 2 * NPAIR - 1:
                                    nc.sync.dma_start(out=out[ts_, dsl], in_=od[:, dt, :])
                            if tb != 2 * NPAIR - 1:
                                nc.sync.dma_start(out=out[ts_, :], in_=od)

                    # o-proj lags one pair behind: its matmuls fill exp-wait
                    # windows and give the ow DMA time to land after phase A.
                    for pi in range(NPAIR):
                        emit_tail(pi, emit_logits_exp(pi))
                        if pi > 0:
                            emit_oproj(pi - 1)
                    emit_oproj(NPAIR - 1)

    nc.compile()
    return nc


_prog = None
last_results = None


def kernel(x, positions, q_w, k_w, v_w, o_w, q_norm_scale, k_norm_scale):
    global _prog, last_results
    x = np.asarray(x); positions = np.asarray(positions)
    q_w = np.asarray(q_w); k_w = np.asarray(k_w); v_w = np.asarray(v_w); o_w = np.asarray(o_w)
    q_norm_scale = np.asarray(q_norm_scale); k_norm_scale = np.asarray(k_norm_scale)

    if _prog is None:
        _prog = _build()
    nc = _prog

    bf = ml_dtypes.bfloat16

    # host-side constants
    j = np.arange(H // 2, dtype=np.float32)
    timescale = (BASE_FREQ ** (2.0 / H * j)).astype(np.float32)

    c = np.arange(128)[:, None]   # key within block (partition)
    r = np.arange(128)[None, :]   # query within block (column)
    up = (c <= r).astype(np.float32)
    lo = (c > r).astype(np.float32)
    masks_np = np.stack([up, lo], axis=1).astype(bf)  # [128, 2, 128]

    scs_np = np.empty((128, 2, 2), np.float32)
    scs_np[:, 0, 0] = 1.0 + q_norm_scale[:128]
    scs_np[:, 0, 1] = 1.0 + q_norm_scale[128:]
    scs_np[:, 1, 0] = 1.0 + k_norm_scale[:128]
    scs_np[:, 1, 1] = 1.0 + k_norm_scale[128:]

    in_maps = []
    for core in range(8):
        b, tp = core // 4, core % 4
        sinu = positions[b].astype(np.float32)[:, None] / timescale[None, :]  # [T, 128]
        qw_h = np.ascontiguousarray(
            q_w[2 * tp:2 * tp + 2].reshape(2, 16, 128, H).transpose(2, 0, 1, 3)).astype(bf)
        kw_h = np.ascontiguousarray(
            k_w[tp].reshape(16, 128, H).transpose(1, 0, 2)).astype(bf)
        vw_h = np.ascontiguousarray(
            v_w[tp].reshape(16, 128, H).transpose(1, 0, 2)).astype(bf)
        ow_h = np.ascontiguousarray(
            o_w[2 * tp:2 * tp + 2].reshape(2, 2, 128, D).transpose(2, 0, 1, 3)).astype(bf)
        # x^T pre-swizzled to quarter-major SBUF layout [p, quarter, d_chunk, t]
        xT_h = np.ascontiguousarray(
            x[b].T.reshape(16, 128, NQ, 512).transpose(1, 2, 0, 3)).astype(bf)
        in_maps.append({
            "xT": xT_h,
            "qw": qw_h,
            "kw": kw_h,
            "vw": vw_h,
            "ow": ow_h,
            "cosT": np.ascontiguousarray(np.cos(sinu).T).astype(np.float32),
            "sinT": np.ascontiguousarray(np.sin(sinu).T).astype(np.float32),
            "masks": masks_np,
            "scs": scs_np,
        })

    res = run_bass_kernel_spmd(nc, in_maps, core_ids=list(range(8)))
    last_results = res

    out = np.zeros((B, T, D), np.float32)
    for core in range(8):
        out[core // 4] += res.results[core]["out"].astype(np.float32)
    return out


# revision 29
# speedup vs baseline: 1.2220x; 1.0011x over previous
"""Sliding-window GQA attention (B=2,T=2048,D=2048,N=8,K=4,H=256,W=1024) on 8 trn2 cores.

Sharding: batch over 2 (fsdp) x heads over 4 (tp). Core (b, tp) computes 2 q heads /
1 kv head for batch b; partial [T, D] outputs are summed over tp on the host.

v2 (bf16): all matmul operands bf16 (fp32 PSUM accumulation) — same PE stream
rate as float32r but FWL halves weight-load time (phase B was LDW-bound), DMA
bytes halve, and SBUF pressure drops. Activation-table thrash eliminated:
phase A ACT = Square+Rsqrt only, phase B ACT = Exp only (1/den moved to the
DVE reciprocal_approx_fast custom op, output copies all on DVE). Host packs
weights in SBUF layout so each weight tensor is one batched DMA. Half-masked
edge key-blocks (j=i+1, j=i-8) only compute their valid 128-query half.

Per-core device pipeline:
  A: qT/kT = W^T x^T (head-dim on partitions) and v (natural layout), streaming
     x^T by 512-token quarters; fused RMS-norm + RoPE out of PSUM.
  B: per 256-token query pair: logits^T = kT^T qT per 128-key block (window
     blocks only), exp on ACT (no max-subtraction: |logit| <= ~6), triangular
     masks on DVE, denominator + P^T V via PE accumulation, divide via
     DVE approx-reciprocal.
  C: out = pvT^T o_w accumulated over local heads, emitted lagged one pair
     behind phase B so its matmuls fill exp-wait windows.
"""
import os

import numpy as np
import ml_dtypes

import concourse.bacc as bacc
import concourse.mybir as mybir
from concourse.tile import TileContext
from concourse.bass_utils import run_bass_kernel_spmd

try:  # pragma: no cover - profiling hook is optional
    from antenv.axon_hooks import get_axon_ntff_profile_hook  # noqa: F401
except ImportError:
    os.environ.setdefault("BASS_NEVER_TRACE", "1")


F32 = mybir.dt.float32
BF = mybir.dt.bfloat16
AF = mybir.ActivationFunctionType
OP = mybir.AluOpType

B, T, D = 2, 2048, 2048
N, KV, H = 8, 4, 256
WINDOW = 1024
BASE_FREQ = 10000.0
EPS = 1e-6
NB = T // 128          # 16 token blocks
NQ = 4                 # t quarters for projections (512 each)
NPAIR = 8              # query-block pairs (256 tokens each)


def _jlist(i):
    return list(range(max(0, i - 8), i + 2))


def _build():
    nc = bacc.Bacc(None)

    xT = nc.dram_tensor("xT", [128, NQ, 16, 512], BF, kind="ExternalInput")
    qw = nc.dram_tensor("qw", [128, 2, 16, 256], BF, kind="ExternalInput")
    kw = nc.dram_tensor("kw", [128, 16, 256], BF, kind="ExternalInput")
    vw = nc.dram_tensor("vw", [128, 16, 256], BF, kind="ExternalInput")
    ow = nc.dram_tensor("ow", [128, 2, 2, D], BF, kind="ExternalInput")
    cosT = nc.dram_tensor("cosT", [128, T], F32, kind="ExternalInput")
    sinT = nc.dram_tensor("sinT", [128, T], F32, kind="ExternalInput")
    masks = nc.dram_tensor("masks", [128, 2, 128], BF, kind="ExternalInput")
    scs = nc.dram_tensor("scs", [128, 2, 2], F32, kind="ExternalInput")  # (1+scale)[q/k][hh]
    out = nc.dram_tensor("out", [T, D], BF, kind="ExternalOutput")

    with TileContext(nc) as tc:
        with tc.tile_pool(name="pers", bufs=1) as pers:
            kT_sb = pers.tile([128, 2, T], BF)
            v_sb = pers.tile([128, NB, H], BF)
            qT_sb = pers.tile([128, 2, 2, T], BF)
            kw_sb = pers.tile([128, 16, 256], BF)
            vw_sb = pers.tile([128, 16, 256], BF)
            qw_sbs = [pers.tile([128, 16, 256], BF, name=f"qw{nl}") for nl in range(2)]
            cos_q = [pers.tile([128, 512], F32, name=f"cos{qt}") for qt in range(NQ)]
            sin_q = [pers.tile([128, 512], F32, name=f"sin{qt}") for qt in range(NQ)]
            scs_sb = pers.tile([128, 2, 2], F32)
            ones32 = pers.tile([128, 128], F32)
            ones = pers.tile([128, 128], BF)
            bias_q = pers.tile([128, 1], F32)
            bias_k = pers.tile([128, 1], F32)
            pvT_sb = pers.tile([128, 2, 2, T], BF)
            ow_sb = pers.tile([128, 2, 2, D], BF)
            masks_sb = pers.tile([128, 2, 128], BF)

            # batched weight DMAs ride the second HWDGE ring (ACT) so they
            # don't queue ahead of the x^T stream on the SP ring. Only what
            # quarter 0's k/v need goes out at t=0 (1.5MB) — the rest is
            # emitted behind phase A ACT ops so its issue is deferred and the
            # x^T stream gets the DMA bandwidth at startup. Quarter 0 runs
            # [k, v, q, q] so the PE has qw-independent work while qw streams.
            nc.scalar.dma_start(out=kw_sb, in_=kw[:, :, :])
            nc.scalar.dma_start(out=vw_sb, in_=vw[:, :, :])
            nc.scalar.dma_start(out=cos_q[0], in_=cosT[:, 0:512])
            nc.scalar.dma_start(out=sin_q[0], in_=sinT[:, 0:512])
            nc.scalar.dma_start(out=scs_sb, in_=scs[:, :, :])
            nc.vector.memset(ones32, 1.0)
            nc.vector.tensor_copy(ones, ones32)
            nc.vector.memset(bias_q, float(H * EPS))
            nc.vector.memset(bias_k, EPS)

            # ---------------- Phase A: all projections + rms + rope ----------------
            with tc.tile_pool(name="xs", bufs=8) as xs, \
                 tc.tile_pool(name="ropep", bufs=1) as ropep, \
                 tc.tile_pool(name="psA", bufs=1, space="PSUM") as psum:

                def rope_emit(p0, p1, dst, kind, cs_t, ss_t):
                    # p0/p1: [128, 512] psum (raw proj h-halves); dst: [128, 2, 512] bf16 view
                    sq0 = ropep.tile([128, 512], BF, tag="sq0", bufs=2)
                    sq1 = ropep.tile([128, 512], BF, tag="sq1", bufs=2)
                    nc.scalar.activation(sq0, p0, AF.Square)
                    nc.scalar.activation(sq1, p1, AF.Square)
                    pss = psum.tile([128, 512], F32, tag="pss", bufs=1)
                    nc.tensor.matmul(pss, ones, sq0, start=True, stop=False)
                    nc.tensor.matmul(pss, ones, sq1, start=False, stop=True)
                    rs = ropep.tile([128, 512], F32, tag="rs", bufs=2)
                    if kind == "q":
                        # 1/16 * rsqrt(ss/256 + eps) == 1/sqrt(ss + 256*eps)
                        nc.scalar.activation(rs, pss, AF.Abs_reciprocal_sqrt,
                                             scale=1.0, bias=bias_q)
                    else:
                        nc.scalar.activation(rs, pss, AF.Abs_reciprocal_sqrt,
                                             scale=1.0 / H, bias=bias_k)
                    cs = ropep.tile([128, 512], F32, tag="cs", bufs=2)
                    ss = ropep.tile([128, 512], F32, tag="ss", bufs=2)
                    nc.vector.tensor_tensor(cs, cs_t, rs, OP.mult)
                    nc.vector.tensor_tensor(ss, ss_t, rs, OP.mult)
                    ki = 0 if kind == "q" else 1
                    s0 = scs_sb[:, ki, 0:1]
                    s1 = scs_sb[:, ki, 1:2]
                    t0 = ropep.tile([128, 512], F32, tag="t0", bufs=2)
                    t1 = ropep.tile([128, 512], F32, tag="t1", bufs=2)
                    nc.vector.scalar_tensor_tensor(t0, p0, s0, cs, OP.mult, OP.mult)
                    nc.vector.scalar_tensor_tensor(t1, p1, s1, ss, OP.mult, OP.mult)
                    nc.vector.tensor_tensor(dst[:, 0, :], t0, t1, OP.subtract)
                    t2 = ropep.tile([128, 512], F32, tag="t0", bufs=2)
                    t3 = ropep.tile([128, 512], F32, tag="t1", bufs=2)
                    nc.vector.scalar_tensor_tensor(t2, p1, s1, cs, OP.mult, OP.mult)
                    nc.vector.scalar_tensor_tensor(t3, p0, s0, ss, OP.mult, OP.mult)
                    nc.vector.tensor_tensor(dst[:, 1, :], t2, t3, OP.add)
                    return rs

                for qt in range(NQ):
                    tq = slice(512 * qt, 512 * (qt + 1))
                    # x^T quarter arrives as 4 batched DMAs of 4 d-chunks each
                    # (fewer DMA_DIRECT2D issues on the sync queue).
                    xqs = []
                    for g in range(4):
                        xq = xs.tile([128, 4, 512], BF, tag="xq")
                        # quarter-major host layout: each group is a contiguous
                        # 4KB run per partition (big DMA descriptors win a fair
                        # round-robin share against the weight ring's packets)
                        nc.sync.dma_start(out=xq, in_=xT[:, qt, slice(4 * g, 4 * g + 4), :])
                        xqs.append(xq)
                    xts = [xqs[d // 4][:, d % 4, :] for d in range(16)]
                    cs_t = cos_q[qt]
                    ss_t = sin_q[qt]

                    def emit_k():
                        # k h-halves interleaved per d-chunk (separate PSUM banks)
                        # so PE consumption keeps pace with the x^T DMA stream.
                        # NB: accumulation groups must NOT interleave in one bank.
                        pk = [psum.tile([128, 512], F32, tag="pq", bufs=6, name=f"pk{qt}_{hh}")
                              for hh in range(2)]
                        for d in range(16):
                            nc.tensor.matmul(pk[0], kw_sb[:, d, 0:128], xts[d],
                                             start=(d == 0), stop=(d == 15))
                            nc.tensor.matmul(pk[1], kw_sb[:, d, 128:256], xts[d],
                                             start=(d == 0), stop=(d == 15))
                        return rope_emit(pk[0], pk[1], kT_sb[:, :, tq], "k", cs_t, ss_t)

                    def emit_q():
                        for nl in range(2):
                            ps = []
                            for hh in range(2):
                                p = psum.tile([128, 512], F32, tag="pq", bufs=6)
                                hs = slice(128 * hh, 128 * (hh + 1))
                                for d in range(16):
                                    nc.tensor.matmul(p, qw_sbs[nl][:, d, hs], xts[d],
                                                     start=(d == 0), stop=(d == 15))
                                ps.append(p)
                            rs = rope_emit(ps[0], ps[1], qT_sb[:, nl, :, tq], "q", cs_t, ss_t)
                        return rs

                    def emit_v():
                        for half in range(2):
                            p = psum.tile([128, 2, H], F32, tag="pva", bufs=1,
                                          name=f"pv{qt}_{half}")
                            tc0 = 4 * qt + 2 * half
                            for sub in range(2):
                                tl = slice(128 * (2 * half + sub), 128 * (2 * half + sub) + 128)
                                for d in range(16):
                                    nc.tensor.matmul(p[:, sub, :], xts[d][:, tl],
                                                     vw_sb[:, d, :],
                                                     start=(d == 0), stop=(d == 15))
                            if qt == NQ - 1:
                                # ACT copy: the DVE queue still holds the last
                                # rope chain; this frees psA sooner at A->B.
                                nc.scalar.copy(v_sb[:, tc0:tc0 + 2, :], p)
                            else:
                                nc.vector.tensor_copy(v_sb[:, tc0:tc0 + 2, :], p)

                    if qt == 0:
                        # v before q: fills the PE while qw still streams in.
                        # The deferred DMA issues sit behind k's rope ACTs on
                        # the scalar queue, so they don't steal bandwidth from
                        # the quarter-0 x^T stream.
                        emit_k()
                        for nl in range(2):
                            nc.scalar.dma_start(out=qw_sbs[nl], in_=qw[:, nl, :, :])
                        emit_v()
                        nc.scalar.dma_start(out=cos_q[1], in_=cosT[:, 512:1024])
                        nc.scalar.dma_start(out=sin_q[1], in_=sinT[:, 512:1024])
                        nc.scalar.dma_start(out=masks_sb, in_=masks[:, :, :])
                        last_rs = emit_q()
                    elif qt == 1:
                        emit_k()
                        nc.scalar.dma_start(out=cos_q[2], in_=cosT[:, 1024:1536])
                        nc.scalar.dma_start(out=sin_q[2], in_=sinT[:, 1024:1536])
                        nc.scalar.dma_start(out=ow_sb, in_=ow[:, :, :, :])
                        last_rs = emit_q()
                        emit_v()
                    elif qt == 2:
                        emit_k()
                        nc.scalar.dma_start(out=cos_q[3], in_=cosT[:, 1536:2048])
                        nc.scalar.dma_start(out=sin_q[3], in_=sinT[:, 1536:2048])
                        last_rs = emit_q()
                        emit_v()
                    else:
                        # v last: its PSUM evacuation is a short DVE copy, so
                        # the psA pool frees quickly at the A->B boundary.
                        emit_k()
                        last_rs = emit_q()
                        emit_v()

                # Preload the Exp activation table while phase A's tail drains
                # (the real first exp would otherwise pay the ~1.3us load right
                # when phase B's denominator is waiting on it). Reading the last
                # rope rsqrt output pins this after all phase A ACT work.
                dummy = ropep.tile([128, 1], BF, tag="dummy", bufs=1)
                nc.scalar.activation(dummy, last_rs[:, 0:1], AF.Exp)

            # ---------------- Phases B + C ----------------
            if True:
                with tc.tile_pool(name="expt", bufs=22) as expt, \
                     tc.tile_pool(name="bw", bufs=4) as bw, \
                     tc.tile_pool(name="oc", bufs=3) as oc, \
                     tc.tile_pool(name="psB", bufs=1, space="PSUM") as psumB:

                    def emit_logits_exp(pi):
                        i = 2 * pi
                        q_lo = slice(256 * pi, 256 * pi + 128)   # first query half
                        q_hi = slice(256 * pi + 128, 256 * (pi + 1))
                        q_all = slice(256 * pi, 256 * (pi + 1))
                        js = _jlist(i)
                        ets = {}
                        up = masks_sb[:, 0, :]
                        lo = masks_sb[:, 1, :]
                        for nl in range(2):
                            for k in range(0, len(js), 2):
                                jp = js[k:k + 2]
                                lp = psumB.tile([128, 2, 256], F32, tag="lp", bufs=4)
                                et = expt.tile([128, 2, 256], BF, tag="et")
                                for x2, j in enumerate(jp):
                                    sj = slice(128 * j, 128 * (j + 1))
                                    if j == i + 1:      # keys above all of q_lo
                                        lps, qsl = lp[:, x2, 128:256], q_hi
                                        ecols = slice(128, 256)
                                    elif j == i - 8:    # keys in-window only for q_lo
                                        lps, qsl = lp[:, x2, 0:128], q_lo
                                        ecols = slice(0, 128)
                                    else:
                                        lps, qsl = lp[:, x2, :], q_all
                                        ecols = slice(0, 256)
                                    nc.tensor.matmul(lps, kT_sb[:, 0, sj],
                                                     qT_sb[:, nl, 0, qsl],
                                                     start=True, stop=False)
                                    nc.tensor.matmul(lps, kT_sb[:, 1, sj],
                                                     qT_sb[:, nl, 1, qsl],
                                                     start=False, stop=True)
                                    ets[(nl, j)] = (et, x2, ecols)
                                # uncomputed edge halves hold stale psum; their
                                # exp lands in et cols no consumer ever reads.
                                nc.scalar.activation(et, lp, AF.Exp)
                                for x2, j in enumerate(jp):
                                    if j == i + 1:
                                        nc.vector.tensor_tensor(
                                            et[:, x2, 128:256], et[:, x2, 128:256], up, OP.mult)
                                    elif j == i:
                                        nc.vector.tensor_tensor(
                                            et[:, x2, 0:128], et[:, x2, 0:128], up, OP.mult)
                                    elif j == i - 7:
                                        nc.vector.tensor_tensor(
                                            et[:, x2, 128:256], et[:, x2, 128:256], lo, OP.mult)
                                    elif j == i - 8:
                                        nc.vector.tensor_tensor(
                                            et[:, x2, 0:128], et[:, x2, 0:128], lo, OP.mult)
                        return ets

                    def emit_tail(pi, ets):
                        i = 2 * pi
                        tqs = slice(256 * pi, 256 * (pi + 1))
                        js = _jlist(i)
                        for nl in range(2):
                            pd = psumB.tile([128, 256], F32, tag="pd", bufs=1)
                            for idx, j in enumerate(js):
                                et, x2, ecols = ets[(nl, j)]
                                nc.tensor.matmul(pd[:, ecols], ones, et[:, x2, ecols],
                                                 start=(idx == 0), stop=(idx == len(js) - 1))
                            rc = bw.tile([128, 256], F32, tag="rc")
                            nc.vector.reciprocal_approx_fast(rc, pd)
                            for hh in range(2):
                                pv = psumB.tile([128, 256], F32, tag="pvb", bufs=3)
                                hs = slice(128 * hh, 128 * (hh + 1))
                                for idx, j in enumerate(js):
                                    et, x2, ecols = ets[(nl, j)]
                                    nc.tensor.matmul(pv[:, ecols], v_sb[:, j, hs],
                                                     et[:, x2, ecols],
                                                     start=(idx == 0), stop=(idx == len(js) - 1))
                                nc.vector.tensor_tensor(pvT_sb[:, nl, hh, tqs], pv, rc, OP.mult)

                    def emit_oproj(pi):
                        # output projection for this pair's two token blocks;
                        # one batched out-DMA per token block (the final block
                        # DMAs per-chunk so the end-of-kernel drain is short).
                        for tb in (2 * pi, 2 * pi + 1):
                            ts_ = slice(128 * tb, 128 * (tb + 1))
                            od = oc.tile([128, 4, 512], BF, tag="od", bufs=3)
                            for dt in range(4):
                                dsl = slice(512 * dt, 512 * (dt + 1))
                                po = psumB.tile([128, 512], F32, tag="pvb", bufs=3)
                                step = 0
                                for nl in range(2):
                                    for hh in range(2):
                                        nc.tensor.matmul(po, pvT_sb[:, nl, hh, ts_],
                                                         ow_sb[:, nl, hh, dsl],
                                                         start=(step == 0), stop=(step == 3))
                                        step += 1
                                nc.vector.tensor_copy(od[:, dt, :], po)
                                if tb == 2 * NPAIR - 1:
                                    # final block: per-chunk DMAs on both HWDGE
                                    # rings so the end-of-kernel receipts overlap
                                    eng = nc.sync if dt % 2 == 0 else nc.scalar
                                    eng.dma_start(out=out[ts_, dsl], in_=od[:, dt, :])
                            if tb != 2 * NPAIR - 1:
                                nc.sync.dma_start(out=out[ts_, :], in_=od)

                    # software-pipelined: logits of pair p+1 cover the exp
                    # latency of pair p's denominator/PV; o-proj lags two pairs
                    # so its matmuls fill exp-wait windows.
                    prev = None
                    for pi in range(NPAIR):
                        cur = emit_logits_exp(pi)
                        if prev is not None:
                            emit_tail(pi - 1, prev)
                        if pi > 1:
                            emit_oproj(pi - 2)
                        prev = cur
                    emit_tail(NPAIR - 1, prev)
                    emit_oproj(NPAIR - 2)
                    emit_oproj(NPAIR - 1)

    nc.compile()
    return nc


_prog = None
last_results = None


def kernel(x, positions, q_w, k_w, v_w, o_w, q_norm_scale, k_norm_scale):
    global _prog, last_results
    x = np.asarray(x); positions = np.asarray(positions)
    q_w = np.asarray(q_w); k_w = np.asarray(k_w); v_w = np.asarray(v_w); o_w = np.asarray(o_w)
    q_norm_scale = np.asarray(q_norm_scale); k_norm_scale = np.asarray(k_norm_scale)

    if _prog is None:
        _prog = _build()
    nc = _prog

    bf = ml_dtypes.bfloat16

    # host-side constants
    j = np.arange(H // 2, dtype=np.float32)
    timescale = (BASE_FREQ ** (2.0 / H * j)).astype(np.float32)

    c = np.arange(128)[:, None]   # key within block (partition)
    r = np.arange(128)[None, :]   # query within block (column)
    up = (c <= r).astype(np.float32)
    lo = (c > r).astype(np.float32)
    masks_np = np.stack([up, lo], axis=1).astype(bf)  # [128, 2, 128]

    scs_np = np.empty((128, 2, 2), np.float32)
    scs_np[:, 0, 0] = 1.0 + q_norm_scale[:128]
    scs_np[:, 0, 1] = 1.0 + q_norm_scale[128:]
    scs_np[:, 1, 0] = 1.0 + k_norm_scale[:128]
    scs_np[:, 1, 1] = 1.0 + k_norm_scale[128:]

    in_maps = []
    for core in range(8):
        b, tp = core // 4, core % 4
        sinu = positions[b].astype(np.float32)[:, None] / timescale[None, :]  # [T, 128]
        qw_h = np.ascontiguousarray(
            q_w[2 * tp:2 * tp + 2].reshape(2, 16, 128, H).transpose(2, 0, 1, 3)).astype(bf)
        kw_h = np.ascontiguousarray(
            k_w[tp].reshape(16, 128, H).transpose(1, 0, 2)).astype(bf)
        vw_h = np.ascontiguousarray(
            v_w[tp].reshape(16, 128, H).transpose(1, 0, 2)).astype(bf)
        ow_h = np.ascontiguousarray(
            o_w[2 * tp:2 * tp + 2].reshape(2, 2, 128, D).transpose(2, 0, 1, 3)).astype(bf)
        # x^T pre-swizzled to quarter-major SBUF layout [p, quarter, d_chunk, t]
        xT_h = np.ascontiguousarray(
            x[b].T.reshape(16, 128, NQ, 512).transpose(1, 2, 0, 3)).astype(bf)
        in_maps.append({
            "xT": xT_h,
            "qw": qw_h,
            "kw": kw_h,
            "vw": vw_h,
            "ow": ow_h,
            "cosT": np.ascontiguousarray(np.cos(sinu).T).astype(np.float32),
            "sinT": np.ascontiguousarray(np.sin(sinu).T).astype(np.float32),
            "masks": masks_np,
            "scs": scs_np,
        })

    res = run_bass_kernel_spmd(nc, in_maps, core_ids=list(range(8)))
    last_results = res

    out = np.zeros((B, T, D), np.float32)
    for core in range(8):
        out[core // 4] += res.results[core]["out"].astype(np.float32)
    return out
